# revision 43
# baseline (speedup 1.0000x reference)
"""AttentionBlock Trainium2 kernel (Bass/Tile), SPMD over 8 NeuronCores.

Problem (hardcoded): x [32, 256, 32, 32] fp32
  GroupNorm(8 groups, eps=1e-5, affine) -> 1x1 qkv conv [768,256] ->
  per-image attention over N=1024 pixels (C=256) -> 1x1 proj [256,256] ->
  residual add.

Sharding: pure data-parallel over batch: 4 images per core, weights
replicated, no collectives.

All matmuls run in fp8(e4m3) with DoubleRow perf mode: the PE array
virtualizes to 128x256, so a full C=256 (or 2-k-block) contraction is
ONE matmul at 2 MAC/cell/cycle. Layouts keep channels split as
[c_lo(partition), c_hi(2), ...] so every operand slices directly into
the required [Ki, 2, free] 3D access pattern:
  - h/q/k stored [P, CT, N] fp8; weights [P, CT, C] fp8; v [P, NB, C] fp8.
  - S^T[k,q] = k^T q unscaled (q,k keep ~N(0,1) range for fp8); the
    1/sqrt(C) scale and a -3 shift fold into the Exp activation
    (exp(S/16 - 3)), keeping exp outputs < 16 (fp8e4 max 240, inf above)
    while the shift cancels between O and Z.
  - exp writes fp8 k-block pairs p[P, 2, 512] (one 1024-wide ACT op per
    pair); O and the Z row accumulate over 4 pair-matmuls per 512-wide
    q chunk in PSUM.
  - GroupNorm stats via bn_stats/bn_aggr (per-channel, fp32, pre-scaled
    by 1/32 with eps folded), pooled per group with a tiny bf16
    mask-matmul, finalized at group level, broadcast back to channels
    via a DRAM-bounce DMA.
  - 1/Z via a [128,4]-transposed reciprocal (DRAM bounce); proj commutes
    with the per-q-column 1/Z scale so proj runs on fp8 O directly and
    the bounce only gates the final DVE multiply-add:
    y = x + proj(O) * (1/Z) + bias.

Work is software-pipelined at emission order (per-engine streams execute
in order): image b+1's x-load/stats run under image b's attention, the
stats finalize (including the group-pool matmul, the only PE op with a
DVE dependency) lands between the two q chunks so it never queues in
front of attention matmuls while waiting, a dummy Exp right after the
per-image Sqrt flips the ACT table back off the critical path, and each
chunk's 1/Z DRAM bounce hides under the next chunk's matmuls. Dep-free
warm-up matmuls after the weight DMAs un-throttle the PE clock (HAM)
before the first real compute.
"""

from contextlib import ExitStack

import ml_dtypes
import numpy as np

import concourse.bass as bass
import concourse.tile as tile
from concourse import bacc
from concourse import mybir

F32 = mybir.dt.float32
BF16 = mybir.dt.bfloat16
F8 = mybir.dt.float8e4
AF = mybir.ActivationFunctionType
OP = mybir.AluOpType
DR = mybir.MatmulPerfMode.DoubleRow

B, C, H, W = 32, 256, 32, 32
N = H * W            # 1024
G = 8                # groups
EPS = 1e-5
NCORES = 8
BL = B // NCORES     # images per core
CT = C // 128        # channel tiles
NB = N // 128        # pixel blocks (k dim of attention)
NP = NB // 2         # k-block pairs per q chunk
QCH = N // 512       # 512-wide q chunks
P = 128
SHIFT = 3.0          # exp(S/16 - SHIFT): keeps fp8 exp outputs < 16
SCALE = float(C) ** -0.5
import os as _os
N_WARM = int(_os.environ.get("KERNEL_N_WARM", "24"))


def build_program(use_bq: bool, use_bk: bool, use_bf: bool) -> bass.Bass:
    nc = bacc.Bacc()

    xs = nc.dram_tensor("xs", [BL, C, N], F32, kind="ExternalInput")
    wq = nc.dram_tensor("wq", [C, C], F8, kind="ExternalInput")  # [c_in, c_out]
    wk = nc.dram_tensor("wk", [C, C], F8, kind="ExternalInput")
    wv = nc.dram_tensor("wv", [C, C], F8, kind="ExternalInput")
    wp = nc.dram_tensor("wp", [C, C], F8, kind="ExternalInput")
    bq = nc.dram_tensor("bq", [C], F32, kind="ExternalInput")
    bk = nc.dram_tensor("bk", [C], F32, kind="ExternalInput")
    bf = nc.dram_tensor("bf", [C], F32, kind="ExternalInput")
    out = nc.dram_tensor("out", [BL, C, N], F32, kind="ExternalOutput")

    # Group indicator for the stat-pooling matmul (chst is pre-scaled by
    # 1/32 on DVE, so plain ones pool to group means).
    gmask_np = np.zeros((P, 4), np.float32)
    gmask_np[np.arange(P), np.arange(P) // 32] = 1.0
    gmask_d = nc.inline_tensor(gmask_np.astype(ml_dtypes.bfloat16), "gmask")
    # Transposed indicator for broadcasting group stats back to their 32
    # channels with a matmul (no DRAM bounce, no sync-queue DMAs).
    bcast_d = nc.inline_tensor(
        np.ascontiguousarray(gmask_np.T).astype(ml_dtypes.bfloat16), "bcast"
    )

    with tile.TileContext(nc) as tc, ExitStack() as ctx:
        consts = ctx.enter_context(tc.tile_pool(name="consts", bufs=1))
        xpool = ctx.enter_context(tc.tile_pool(name="xp", bufs=3))
        hpool = ctx.enter_context(tc.tile_pool(name="hp", bufs=2))
        qpool = ctx.enter_context(tc.tile_pool(name="qp", bufs=2))
        kpool = ctx.enter_context(tc.tile_pool(name="kp", bufs=2))
        vpool = ctx.enter_context(tc.tile_pool(name="vp", bufs=2))
        ppool = ctx.enter_context(tc.tile_pool(name="pp", bufs=3))
        opool = ctx.enter_context(tc.tile_pool(name="op", bufs=2))
        spool = ctx.enter_context(tc.tile_pool(name="sp", bufs=2))
        rzpool = ctx.enter_context(tc.tile_pool(name="rzp", bufs=2))
        outp = ctx.enter_context(tc.tile_pool(name="outp", bufs=4))
        dram = ctx.enter_context(tc.tile_pool(name="dram", bufs=2, space="DRAM"))
        # PSUM: "w" slots ([P,2,512] = 2 banks) x2 serve the S pairs and
        # every qkv/proj/warmup/stat matmul; O accumulators 2 banks; z
        # rows 2 banks. 4 + 2 + 2 = 8 banks exactly.
        psA = ctx.enter_context(tc.tile_pool(name="psA", bufs=2, space="PSUM"))
        psO = ctx.enter_context(tc.tile_pool(name="psO", bufs=1, space="PSUM"))
        psz = ctx.enter_context(tc.tile_pool(name="psz", bufs=2, space="PSUM"))

        # --- constants (weights DMA first: warmups gate on them) ---
        wq_sb = consts.tile([P, CT, C], F8, tag="wq")
        wk_sb = consts.tile([P, CT, C], F8, tag="wk")
        wv_sb = consts.tile([P, CT, C], F8, tag="wv")
        wp_sb = consts.tile([P, CT, C], F8, tag="wp")
        for t_sb, t_d in ((wq_sb, wq), (wk_sb, wk), (wv_sb, wv), (wp_sb, wp)):
            nc.sync.dma_start(
                out=t_sb, in_=t_d[:, :].rearrange("(t p) o -> p t o", p=P)
            )
        gmask_sb = consts.tile([P, 4], BF16, tag="gmask")
        nc.sync.dma_start(out=gmask_sb, in_=gmask_d[:, :])
        bcast_sb = consts.tile([4, P], BF16, tag="bcast")
        nc.sync.dma_start(out=bcast_sb, in_=bcast_d[:, :])
        bq_sb = consts.tile([P, CT], F32, tag="bq")
        nc.sync.dma_start(out=bq_sb, in_=bq[:].rearrange("(t p) -> p t", p=P))
        bk_sb = consts.tile([P, CT], F32, tag="bk")
        nc.sync.dma_start(out=bk_sb, in_=bk[:].rearrange("(t p) -> p t", p=P))
        bf_sb = consts.tile([P, CT], F32, tag="bf")
        nc.sync.dma_start(out=bf_sb, in_=bf[:].rearrange("(t p) -> p t", p=P))
        # fp8 ones pairs for the Z row matmul; 16-wide so the middle-dim
        # byte stride satisfies the DoubleRow AP constraint (step%16==0).
        onesp_sb = consts.tile([P, 2, 16], F8, tag="onesp")
        nc.vector.memset(onesp_sb, 1.0)
        shift_sb = consts.tile([P, 1], F32, tag="shift")
        nc.vector.memset(shift_sb, -SHIFT)
        # Const tiles for the DVE fast-inverse-sqrt (keeps ScalarE's
        # activation table pinned to EXP for the whole kernel).
        I32 = mybir.dt.int32
        magic_sb = consts.tile([4, CT], I32, tag="magic")
        nc.vector.memset(magic_sb, 0x5F3759DF)
        cone_sb = consts.tile([4, CT], I32, tag="cone")
        nc.vector.memset(cone_sb, 1)
        chalf_sb = consts.tile([4, CT], F32, tag="chalf")
        nc.vector.memset(chalf_sb, 0.5)
        c15_sb = consts.tile([4, CT], F32, tag="c15")
        nc.vector.memset(c15_sb, 1.5)
        # Per-image state carried between pipeline phases.
        st = [dict() for _ in range(BL)]

        def phase_a_load(b):
            """Load x; per-channel stats on DVE, pre-scaled for pooling:
            chst[:,0,ct] = mean/32, chst[:,1,ct] = (E[x^2] + eps)/32."""
            x_t = xpool.tile([P, CT, N], F32, tag="x")
            st[b]["x"] = x_t
            for ct in range(CT):
                for s in range(2):
                    nc.gpsimd.dma_start(
                        out=x_t[:, ct, s * 512 : (s + 1) * 512],
                        in_=xs[b, ct * P : (ct + 1) * P, s * 512 : (s + 1) * 512],
                    )
            chst = spool.tile([P, 2, CT], F32, tag="chst")
            st[b]["chst"] = chst
            for ct in range(CT):
                bnst = spool.tile([P, 2, 6], F32, tag="bnst")
                for s in range(2):
                    nc.vector.bn_stats(
                        out=bnst[:, s, :], in_=x_t[:, ct, s * 512 : (s + 1) * 512]
                    )
                nc.vector.bn_aggr(out=chst[:, :, ct], in_=bnst)
            msq = spool.tile([P, CT], F32, tag="msq")
            nc.vector.tensor_mul(out=msq, in0=chst[:, 0, :], in1=chst[:, 0, :])
            nc.vector.tensor_add(out=chst[:, 1, :], in0=chst[:, 1, :], in1=msq)
            nc.vector.tensor_scalar(
                out=chst[:, 1, :], in0=chst[:, 1, :],
                scalar1=1.0 / 32, scalar2=EPS / 32, op0=OP.mult, op1=OP.add,
            )
            nc.vector.tensor_scalar_mul(
                out=chst[:, 0, :], in0=chst[:, 0, :], scalar1=1.0 / 32
            )

        def phase_a_stats(b):
            """Group pooling matmul, group rstd (fast-inverse-sqrt)."""
            chst = st[b].pop("chst")
            chst_bf = spool.tile([P, 2 * CT], BF16, tag="chstbf")
            nc.vector.tensor_copy(
                out=chst_bf, in_=chst.rearrange("p a c -> p (a c)")
            )
            gst_ps = psA.tile([4, 2 * CT], F32, tag="w", name="gst_ps")
            nc.tensor.matmul(
                gst_ps, lhsT=gmask_sb, rhs=chst_bf, start=True, stop=True
            )
            # gst rows: [gmean ct0, gmean ct1, gE2e ct0, gE2e ct1] on
            # partitions 0-3.
            gst_sb = spool.tile([4, 2 * CT], F32, tag="gst")
            nc.vector.tensor_copy(out=gst_sb, in_=gst_ps)
            gvar = spool.tile([4, CT], F32, tag="gvar")
            nc.vector.tensor_mul(
                out=gvar, in0=gst_sb[:, 0:CT], in1=gst_sb[:, 0:CT]
            )
            nc.vector.tensor_sub(out=gvar, in0=gst_sb[:, CT:], in1=gvar)
            # rstd = 1/sqrt(var+eps) via fast-inverse-sqrt on DVE (bit
            # seed + 2 Newton steps, ~1e-5 rel): ScalarE keeps its EXP
            # table for the whole kernel (a table swap costs ~1.5us and
            # would land right ahead of the next chunk's exps).
            yi = spool.tile([4, CT], mybir.dt.int32, tag="yi")
            nc.vector.tensor_tensor(
                out=yi, in0=gvar.bitcast(mybir.dt.int32), in1=cone_sb,
                op=OP.logical_shift_right,
            )
            grstd = spool.tile([4, CT], F32, tag="grstd")
            nc.vector.tensor_tensor(
                out=grstd.bitcast(mybir.dt.int32), in0=magic_sb, in1=yi,
                op=OP.subtract,
            )
            nt = spool.tile([4, CT], F32, tag="nt")
            for _ in range(2):
                nc.vector.tensor_mul(out=nt, in0=grstd, in1=grstd)
                nc.vector.tensor_mul(out=nt, in0=nt, in1=gvar)
                nc.vector.tensor_mul(out=nt, in0=nt, in1=chalf_sb)
                nc.vector.tensor_sub(out=nt, in0=c15_sb, in1=nt)
                nc.vector.tensor_mul(out=grstd, in0=grstd, in1=nt)
            # Broadcast (mean, rstd) back to each group's 32 channels
            # with a tiny mask-matmul (bf16 stats; ~0.4% on rstd is far
            # below the fp8 h quantization).
            gfin = spool.tile([4, 2, CT], BF16, tag="gfin")
            nc.vector.tensor_copy(out=gfin[:, 0, :], in_=gst_sb[:, 0:CT])
            nc.vector.tensor_copy(out=gfin[:, 1, :], in_=grstd)
            st[b]["gfin"] = gfin

        def phase_a_pcs(b):
            """Per-channel stats broadcast + h cast."""
            gfin = st[b].pop("gfin")
            x_t = st[b]["x"]
            pcs_ps = psA.tile([P, 2 * CT], F32, tag="w", name="pcs_ps")
            nc.tensor.matmul(
                pcs_ps, lhsT=bcast_sb, rhs=gfin.rearrange("p a c -> p (a c)"),
                start=True, stop=True,
            )
            pcs = spool.tile([P, 2, CT], F32, tag="pcs")
            nc.vector.tensor_copy(
                out=pcs.rearrange("p a c -> p (a c)"), in_=pcs_ps
            )
            h_t = hpool.tile([P, CT, N], F8, tag="h")
            st[b]["h"] = h_t
            for ct in range(CT):
                nc.vector.tensor_scalar(
                    out=h_t[:, ct, :],
                    in0=x_t[:, ct, :],
                    scalar1=pcs[:, 0, ct : ct + 1],
                    scalar2=pcs[:, 1, ct : ct + 1],
                    op0=OP.subtract,
                    op1=OP.mult,
                )

        def phase_b(b):
            """qkv 1x1 convs (fp8 DoubleRow, full C contraction per mm).

            One single-bank psum tile per matmul; q casts on ScalarE
            (Copy is table-free) run in parallel with k/v casts on DVE.
            """
            h_t = st[b]["h"]
            q_sb = qpool.tile([P, CT, N], F8, tag="q")
            k_sb = kpool.tile([P, CT, N], F8, tag="k")
            st[b]["q"], st[b]["k"] = q_sb, k_sb
            for dst, w_sb, b_sb, use_b, on_act in (
                (q_sb, wq_sb, bq_sb, use_bq, True),
                (k_sb, wk_sb, bk_sb, use_bk, False),
            ):
                for ct in range(CT):
                    for nch in range(2):
                        mm_ps = psA.tile([P, 512], F32, tag="w", name="qk_ps")
                        nc.tensor.matmul(
                            mm_ps,
                            lhsT=w_sb[:, :, ct * P : (ct + 1) * P],
                            rhs=h_t[:, :, nch * 512 : (nch + 1) * 512],
                            start=True,
                            stop=True,
                            perf_mode=DR,
                        )
                        dst_ap = dst[:, ct, nch * 512 : (nch + 1) * 512]
                        if use_b:
                            nc.vector.tensor_scalar_add(
                                out=dst_ap, in0=mm_ps, scalar1=b_sb[:, ct : ct + 1]
                            )
                        elif on_act or ct == 0:
                            # q fully + k's first ct on ScalarE (Copy is
                            # table-free) so the DVE k/v chain shortens.
                            nc.scalar.activation(
                                out=dst_ap, in_=mm_ps, func=AF.Copy, bias=0.0,
                                scale=1.0,
                            )
                        else:
                            nc.vector.tensor_copy(out=dst_ap, in_=mm_ps)
            v_sb = vpool.tile([P, NB, C], F8, tag="v")
            st[b]["v"] = v_sb
            for nb in range(NB):
                vv_ps = psA.tile([P, C], F32, tag="w", name="vv_ps")
                nc.tensor.matmul(
                    vv_ps,
                    lhsT=h_t[:, :, nb * P : (nb + 1) * P],
                    rhs=wv_sb[:, :, :],
                    start=True,
                    stop=True,
                    perf_mode=DR,
                )
                nc.vector.tensor_copy(out=v_sb[:, nb, :], in_=vv_ps)

        def phase_c(b, qc, mid=None):
            """Attention core for one 512-wide q chunk: S, exp, Z, O.

            `mid` (the previous chunk's phase_d) is emitted after the
            pair loop: its proj matmuls fill the PE while this chunk's
            tail exps drain, and its psum slot is reused two S-pairs
            later (by which point the 1/Z bounce has long completed).
            """
            q_sb, k_sb, v_sb = st[b]["q"], st[b]["k"], st[b]["v"]
            O_ps = psO.tile([P, CT, 512], F32, tag="O")
            z_ps = psz.tile([1, 512], F32, tag="z")
            st[b]["zps%d" % qc] = z_ps

            def s_pair(j):
                sp = psA.tile([P, 2, 512], F32, tag="w", name="s_pair")
                for t in range(2):
                    nc.tensor.matmul(
                        sp[:, t, :],
                        lhsT=k_sb[:, :, (2 * j + t) * P : (2 * j + t + 1) * P],
                        rhs=q_sb[:, :, qc * 512 : (qc + 1) * 512],
                        start=True,
                        stop=True,
                        perf_mode=DR,
                    )
                return sp

            fifo = [s_pair(0), s_pair(1)]
            for j in range(NP):
                sp = fifo.pop(0)
                p_cur = ppool.tile([P, 2, 512], F8, tag="p")
                nc.scalar.activation(
                    out=p_cur, in_=sp, func=AF.Exp, bias=shift_sb, scale=SCALE,
                )
                if j + 2 < NP:
                    fifo.append(s_pair(j + 2))
                nc.tensor.matmul(
                    z_ps,
                    lhsT=onesp_sb[:, :, 0:1],
                    rhs=p_cur,
                    start=(j == 0),
                    stop=(j == NP - 1),
                    perf_mode=DR,
                )
                for ct in range(CT):
                    nc.tensor.matmul(
                        O_ps[:, ct, :],
                        lhsT=v_sb[:, 2 * j : 2 * j + 2, ct * P : (ct + 1) * P],
                        rhs=p_cur,
                        start=(j == 0),
                        stop=(j == NP - 1),
                        perf_mode=DR,
                    )
            # Z leaves PSUM on ScalarE right at the chunk end so the
            # psz slot frees early and the 1/Z chain starts immediately.
            z_sb = rzpool.tile([1, 512], F32, tag="zsb")
            st[b]["zsb%d" % qc] = z_sb
            nc.scalar.activation(
                out=z_sb, in_=z_ps, func=AF.Copy, bias=0.0, scale=1.0
            )
            if mid is not None:
                mid()
            # proj commutes with the per-column 1/Z scale, so proj depends
            # only on O: cast O out of PSUM per ct (releasing the O banks
            # a chunk early, split across ScalarE/DVE); the 1/Z bounce
            # gates just the final multiply-add.
            on_sb = opool.tile([P, CT, 512], F8, tag="on")
            st[b]["on%d" % qc] = on_sb
            nc.scalar.activation(
                out=on_sb[:, 0, :], in_=O_ps[:, 0, :], func=AF.Copy, bias=0.0,
                scale=1.0,
            )
            nc.vector.tensor_copy(out=on_sb[:, 1, :], in_=O_ps[:, 1, :])

        def phase_rz(b, qc):
            z_sb = st[b].pop("zsb%d" % qc)
            st[b].pop("zps%d" % qc)
            # 1/Z with the row transposed to [128, 4] so the reciprocal
            # runs across lanes (a [1, 512] reciprocal costs ~4us on DVE).
            # Both legs bounce through DRAM: the direct SBUF->SBUF
            # redistribute misroutes on hardware (sim-only correct).
            z_d = dram.tile([1, 512], F32, tag="zd")
            nc.sync.dma_start(out=z_d, in_=z_sb)
            zT_sb = rzpool.tile([P, 4], F32, tag="zT")
            nc.sync.dma_start(
                out=zT_sb, in_=z_d[0, :].rearrange("(p j) -> p j", j=4)
            )
            rzT_sb = rzpool.tile([P, 4], F32, tag="rzT")
            nc.vector.reciprocal(out=rzT_sb, in_=zT_sb)
            rz_d = dram.tile([1, 512], F32, tag="rzd")
            nc.sync.dma_start(
                out=rz_d[0, :].rearrange("(p j) -> p j", j=4), in_=rzT_sb
            )
            rzb_sb = rzpool.tile([P, 512], F32, tag="rzb")
            st[b]["rzb%d" % qc] = rzb_sb
            nc.sync.dma_start(out=rzb_sb, in_=rz_d[:, :].to_broadcast((P, 512)))

        def phase_d(b, qc):
            """Apply 1/Z on the fp8 O, proj conv, residual add, store.

            The per-q-column 1/Z scale commutes with proj, so it lands on
            on_sb BEFORE the matmuls: the proj psum slot's only consumer
            is then a dependency-free DVE add (the 1/Z bounce can never
            stall the attention S-pairs through the psum ring).
            """
            rzb_sb = st[b].pop("rzb%d" % qc)
            x_t = st[b]["x"]
            on_sb = st[b].pop("on%d" % qc)
            pr_ps = psA.tile([P, 2, 512], F32, tag="w", name="pr_ps")
            for ct in range(CT):
                nc.tensor.matmul(
                    pr_ps[:, ct, :],
                    lhsT=wp_sb[:, :, ct * P : (ct + 1) * P],
                    rhs=on_sb[:, :, :],
                    start=True,
                    stop=True,
                    perf_mode=DR,
                )
            o_sb = outp.tile([P, 2, 512], F32, tag="o")
            for ct in range(CT):
                xres = x_t[:, ct, qc * 512 : (qc + 1) * 512]
                nc.vector.tensor_mul(
                    out=o_sb[:, ct, :], in0=pr_ps[:, ct, :], in1=rzb_sb
                )
                if use_bf:
                    nc.gpsimd.scalar_tensor_tensor(
                        out=o_sb[:, ct, :],
                        in0=o_sb[:, ct, :],
                        scalar=bf_sb[:, ct : ct + 1],
                        in1=xres,
                        op0=OP.add,
                        op1=OP.add,
                    )
                else:
                    nc.gpsimd.tensor_add(
                        out=o_sb[:, ct, :], in0=o_sb[:, ct, :], in1=xres
                    )
            nc.gpsimd.dma_start(
                out=out[b].rearrange("(t p) n -> p t n", p=P)[
                    :, :, qc * 512 : (qc + 1) * 512
                ],
                in_=o_sb,
            )

        # Software pipeline: image b+1's x-load/stats hide under image
        # b's attention, the stats finalize lands between the two q
        # chunks, and each chunk's 1/Z bounce hides under the next
        # chunk's matmuls.
        phase_a_load(0)
        for _ in range(N_WARM):
            warm_ps = psA.tile([P, 512], F32, tag="w", name="warm_ps")
            nc.tensor.matmul(
                warm_ps[:, 0:256], lhsT=wq_sb[:, 0, 0:P],
                rhs=wq_sb[:, 0, 0:256], start=True, stop=True,
            )
        phase_a_stats(0)
        phase_a_pcs(0)
        pending = None
        for b in range(BL):
            phase_b(b)
            if b + 1 < BL:
                phase_a_load(b + 1)
            prev = pending
            phase_c(b, 0, mid=(lambda p=prev: phase_d(*p)) if prev else None)
            phase_rz(b, 0)
            if b + 1 < BL:
                phase_a_stats(b + 1)
                phase_a_pcs(b + 1)
            phase_c(b, 1, mid=lambda: phase_d(b, 0))
            phase_rz(b, 1)
            pending = (b, 1)
        phase_d(*pending)
    nc.compile()
    return nc


def prepare(inputs):
    """Fold parameters on the host; return (program, per-core input maps)."""
    x = np.ascontiguousarray(np.asarray(inputs["x"], dtype=np.float32))
    norm_w = np.asarray(inputs["norm_w"], dtype=np.float32)
    norm_b = np.asarray(inputs["norm_b"], dtype=np.float32)
    qkv_w = np.asarray(inputs["qkv_w"], dtype=np.float32)
    qkv_b = np.asarray(inputs["qkv_b"], dtype=np.float32)
    proj_w = np.asarray(inputs["proj_w"], dtype=np.float32)
    proj_b = np.asarray(inputs["proj_b"], dtype=np.float32)

    # Fold the GroupNorm affine into qkv: qkv(h*w+b) = (qkv*w)h + qkv@b
    w_eff = qkv_w * norm_w[None, :]
    b_eff = qkv_b + qkv_w @ norm_b
    fp8 = ml_dtypes.float8_e4m3

    def to8(a):
        return np.ascontiguousarray(np.clip(a, -240.0, 240.0).astype(fp8))

    # q/k stay unscaled (~N(0,1) is the fp8 sweet spot); the attention
    # 1/sqrt(C) scale is applied inside the Exp activation on-chip.
    wq_t = to8(w_eff[0:C].T)
    wk_t = to8(w_eff[C : 2 * C].T)
    wv_t = to8(w_eff[2 * C : 3 * C].T)
    wp_t = to8(proj_w.T)
    bq_f = np.ascontiguousarray(b_eff[0:C])
    bk_f = np.ascontiguousarray(b_eff[C : 2 * C])
    bv_f = b_eff[2 * C : 3 * C]
    bf_f = np.ascontiguousarray(proj_w @ bv_f + proj_b)

    use_bq = bool(np.any(bq_f))
    use_bk = bool(np.any(bk_f))
    use_bf = bool(np.any(bf_f))
    nc = build_program(use_bq, use_bk, use_bf)

    xr = x.reshape(NCORES, BL, C, N)
    in_maps = []
    for c in range(NCORES):
        in_maps.append(
            {
                "xs": np.ascontiguousarray(xr[c]),
                "wq": wq_t,
                "wk": wk_t,
                "wv": wv_t,
                "wp": wp_t,
                "bq": bq_f,
                "bk": bk_f,
                "bf": bf_f,
            }
        )
    return nc, in_maps


def run(inputs, trace=False):
    from concourse.bass_utils import run_bass_kernel_spmd

    nc, in_maps = prepare(inputs)
    res = run_bass_kernel_spmd(nc, in_maps, list(range(NCORES)), trace=trace)
    outs = np.stack([np.asarray(res.results[i]["out"]) for i in range(NCORES)])
    full = outs.reshape(B, C, H, W).astype(np.float32)
    return full, res


def kernel(**inputs) -> np.ndarray:
    full, _ = run(inputs, trace=False)
    return full


# revision 44
# speedup vs baseline: 1.0537x; 1.0537x over previous
"""AttentionBlock Trainium2 kernel (Bass/Tile), SPMD over 8 NeuronCores.

Problem (hardcoded): x [32, 256, 32, 32] fp32
  GroupNorm(8 groups, eps=1e-5, affine) -> 1x1 qkv conv [768,256] ->
  per-image attention over N=1024 pixels (C=256) -> 1x1 proj [256,256] ->
  residual add.

Sharding: pure data-parallel over batch: 4 images per core, weights
replicated, no collectives.

All matmuls run in fp8(e4m3) with DoubleRow perf mode: the PE array
virtualizes to 128x256, so a full C=256 (or 2-k-block) contraction is
ONE matmul at 2 MAC/cell/cycle (~259 ns warm for a 512-wide output vs
~2x426 ns in bf16). Layouts keep channels split as [c_lo(partition),
c_hi(2), ...] so every operand slices directly into the required
[Ki, 2, free] 3D access pattern:
  - h/q/k stored [P, CT, N] fp8; weights [P, CT, C] fp8; v [P, NB, C] fp8.
  - S^T[k,q] = k^T q unscaled (q,k keep ~N(0,1) range for fp8); the
    1/sqrt(C) scale and a -3 shift fold into the Exp activation
    (exp(S/16 - 3)), keeping exp outputs < 16 (fp8e4 max 240, inf above)
    while the shift cancels exactly between O and Z.
  - exp writes fp8 k-block pairs p[P, 2, 512]; O and the Z row
    accumulate over 4 pair-matmuls per 512-wide q chunk in PSUM.
  - GroupNorm stats via bn_stats/bn_aggr (per-channel, fp32), pooled
    over each group's 32 channels with a tiny bf16 mask-matmul,
    finalized at group level in fp32, broadcast back to channels via a
    DRAM-bounce DMA.
  - 1/Z via a [128,4]-transposed reciprocal (DRAM bounce; the direct
    SBUF->SBUF redistribute silently misroutes on hardware); proj
    commutes with the per-q-column 1/Z scale, so proj runs directly on
    the fp8 O and the bounce only gates the final DVE multiply-add:
    y = x + proj(O) * (1/Z) + bias.

The per-image work is software-pipelined at emission order (per-engine
instruction streams execute in order): image b+1's x-load/stats run on
DVE under image b's attention matmuls, the S fifo keeps 3 tiles in
flight so the PE rarely waits on ScalarE's exp, each q-chunk's 1/Z DRAM
bounce hides under the next chunk's matmuls (phase_d deferred by one
chunk), and the 4-deep single-bank psum ring for S/qkv/proj keeps slot
antidependencies loose. Dep-free warm-up matmuls after the weight DMAs
un-throttle the PE clock (HAM) before the first real compute.

Measured on 8 axon trn2 cores: ~153-190 us HW exec (run-to-run HAM/
scheduling variance), rel err ~6.5e-3.
"""

from contextlib import ExitStack

import ml_dtypes
import numpy as np

import concourse.bass as bass
import concourse.tile as tile
from concourse import bacc
from concourse import mybir

F32 = mybir.dt.float32
BF16 = mybir.dt.bfloat16
F8 = mybir.dt.float8e4
AF = mybir.ActivationFunctionType
OP = mybir.AluOpType
DR = mybir.MatmulPerfMode.DoubleRow

B, C, H, W = 32, 256, 32, 32
N = H * W            # 1024
G = 8                # groups
EPS = 1e-5
NCORES = 8
BL = B // NCORES     # images per core
CT = C // 128        # channel tiles
NB = N // 128        # pixel blocks (k dim of attention)
NP = NB // 2         # k-block pairs per q chunk
QCH = N // 512       # 512-wide q chunks
P = 128
SHIFT = 3.0          # exp(S/16 - SHIFT): keeps fp8 exp outputs < 16
SCALE = float(C) ** -0.5
import os as _os
N_WARM = int(_os.environ.get("KERNEL_N_WARM", "24"))
S_FIFO = int(_os.environ.get("KERNEL_S_FIFO", "3"))


def build_program(use_bq: bool, use_bk: bool, use_bf: bool) -> bass.Bass:
    nc = bacc.Bacc()

    xs = nc.dram_tensor("xs", [BL, C, N], F32, kind="ExternalInput")
    wq = nc.dram_tensor("wq", [C, C], F8, kind="ExternalInput")  # [c_in, c_out]
    wk = nc.dram_tensor("wk", [C, C], F8, kind="ExternalInput")
    wv = nc.dram_tensor("wv", [C, C], F8, kind="ExternalInput")
    wp = nc.dram_tensor("wp", [C, C], F8, kind="ExternalInput")
    bq = nc.dram_tensor("bq", [C], F32, kind="ExternalInput")
    bk = nc.dram_tensor("bk", [C], F32, kind="ExternalInput")
    bf = nc.dram_tensor("bf", [C], F32, kind="ExternalInput")
    out = nc.dram_tensor("out", [BL, C, N], F32, kind="ExternalOutput")

    # Constant matrix for the group-stat pooling matmul (mean over each
    # group's 32 channels; 1/32 is exact in bf16).
    gmask_np = np.zeros((P, 4), np.float32)
    gmask_np[np.arange(P), np.arange(P) // 32] = 1.0 / 32.0
    gmask_d = nc.inline_tensor(gmask_np.astype(ml_dtypes.bfloat16), "gmask")

    with tile.TileContext(nc) as tc, ExitStack() as ctx:
        consts = ctx.enter_context(tc.tile_pool(name="consts", bufs=1))
        xpool = ctx.enter_context(tc.tile_pool(name="xp", bufs=3))
        hpool = ctx.enter_context(tc.tile_pool(name="hp", bufs=2))
        qpool = ctx.enter_context(tc.tile_pool(name="qp", bufs=2))
        kpool = ctx.enter_context(tc.tile_pool(name="kp", bufs=2))
        vpool = ctx.enter_context(tc.tile_pool(name="vp", bufs=2))
        ppool = ctx.enter_context(tc.tile_pool(name="pp", bufs=3))
        opool = ctx.enter_context(tc.tile_pool(name="op", bufs=2))
        spool = ctx.enter_context(tc.tile_pool(name="sp", bufs=2))
        rzpool = ctx.enter_context(tc.tile_pool(name="rzp", bufs=2))
        outp = ctx.enter_context(tc.tile_pool(name="outp", bufs=4))
        dram = ctx.enter_context(tc.tile_pool(name="dram", bufs=2, space="DRAM"))
        psw = ctx.enter_context(tc.tile_pool(name="psw", bufs=4, space="PSUM"))
        psO = ctx.enter_context(tc.tile_pool(name="psO", bufs=1, space="PSUM"))
        psz = ctx.enter_context(tc.tile_pool(name="psz", bufs=2, space="PSUM"))

        # --- constants ---
        gmask_sb = consts.tile([P, 4], BF16, tag="gmask")
        nc.sync.dma_start(out=gmask_sb, in_=gmask_d[:, :])
        bq_sb = consts.tile([P, CT], F32, tag="bq")
        nc.sync.dma_start(out=bq_sb, in_=bq[:].rearrange("(t p) -> p t", p=P))
        bk_sb = consts.tile([P, CT], F32, tag="bk")
        nc.sync.dma_start(out=bk_sb, in_=bk[:].rearrange("(t p) -> p t", p=P))
        bf_sb = consts.tile([P, CT], F32, tag="bf")
        nc.sync.dma_start(out=bf_sb, in_=bf[:].rearrange("(t p) -> p t", p=P))
        # fp8 ones pairs for the Z row matmul; 16-wide so the middle-dim
        # byte stride satisfies the DoubleRow AP constraint (step%16==0).
        onesp_sb = consts.tile([P, 2, 16], F8, tag="onesp")
        nc.vector.memset(onesp_sb, 1.0)
        eps_sb = consts.tile([P, 1], F32, tag="eps")
        nc.vector.memset(eps_sb, EPS)
        shift_sb = consts.tile([P, 1], F32, tag="shift")
        nc.vector.memset(shift_sb, -SHIFT)
        wq_sb = consts.tile([P, CT, C], F8, tag="wq")
        wk_sb = consts.tile([P, CT, C], F8, tag="wk")
        wv_sb = consts.tile([P, CT, C], F8, tag="wv")
        wp_sb = consts.tile([P, CT, C], F8, tag="wp")

        def load_weights():
            for t_sb, t_d in ((wq_sb, wq), (wk_sb, wk), (wv_sb, wv), (wp_sb, wp)):
                nc.sync.dma_start(
                    out=t_sb, in_=t_d[:, :].rearrange("(t p) o -> p t o", p=P)
                )

        # Per-image state carried between pipeline phases.
        st = [dict() for _ in range(BL)]

        def phase_a(b):
            """Load x, GroupNorm stats -> per-channel (mean, rstd), h."""
            x_t = xpool.tile([P, CT, N], F32, tag="x")
            st[b]["x"] = x_t
            for ct in range(CT):
                nc.sync.dma_start(
                    out=x_t[:, ct, :], in_=xs[b, ct * P : (ct + 1) * P, :]
                )
            chst = spool.tile([P, 2 * CT], F32, tag="chst")
            for ct in range(CT):
                bnst = spool.tile([P, 2, 6], F32, tag="bnst")
                for s in range(2):
                    nc.vector.bn_stats(
                        out=bnst[:, s, :], in_=x_t[:, ct, s * 512 : (s + 1) * 512]
                    )
                nc.vector.bn_aggr(out=chst[:, 2 * ct : 2 * ct + 2], in_=bnst)
                msq = spool.tile([P, 1], F32, tag="msq")
                nc.vector.tensor_mul(
                    out=msq,
                    in0=chst[:, 2 * ct : 2 * ct + 1],
                    in1=chst[:, 2 * ct : 2 * ct + 1],
                )
                nc.vector.tensor_add(
                    out=chst[:, 2 * ct + 1 : 2 * ct + 2],
                    in0=chst[:, 2 * ct + 1 : 2 * ct + 2],
                    in1=msq,
                )
            chst_bf = spool.tile([P, 2 * CT], BF16, tag="chstbf")
            nc.vector.tensor_copy(out=chst_bf, in_=chst)
            gst_ps = psw.tile([4, 2 * CT], F32, tag="w")
            nc.tensor.matmul(
                gst_ps, lhsT=gmask_sb, rhs=chst_bf, start=True, stop=True
            )
            gst_sb = spool.tile([4, 2 * CT], F32, tag="gst")
            nc.vector.tensor_copy(out=gst_sb, in_=gst_ps)
            gvar = spool.tile([4, CT], F32, tag="gvar")
            for ct in range(CT):
                gmsq = spool.tile([4, 1], F32, tag="gmsq")
                nc.vector.tensor_mul(
                    out=gmsq,
                    in0=gst_sb[:, 2 * ct : 2 * ct + 1],
                    in1=gst_sb[:, 2 * ct : 2 * ct + 1],
                )
                nc.vector.tensor_tensor(
                    out=gvar[:, ct : ct + 1],
                    in0=gst_sb[:, 2 * ct + 1 : 2 * ct + 2],
                    in1=gmsq,
                    op=OP.subtract,
                )
            gsd = spool.tile([4, CT], F32, tag="gsd")
            nc.scalar.activation(
                out=gsd, in_=gvar, func=AF.Sqrt, bias=eps_sb[0:4], scale=1.0
            )
            grstd = spool.tile([4, CT], F32, tag="grstd")
            nc.vector.reciprocal(out=grstd, in_=gsd)
            gfin = spool.tile([4, 2 * CT], F32, tag="gfin")
            for ct in range(CT):
                nc.vector.tensor_copy(
                    out=gfin[:, 2 * ct : 2 * ct + 1],
                    in_=gst_sb[:, 2 * ct : 2 * ct + 1],
                )
                nc.vector.tensor_copy(
                    out=gfin[:, 2 * ct + 1 : 2 * ct + 2],
                    in_=grstd[:, ct : ct + 1],
                )
            gfin_d = dram.tile([4, 2 * CT], F32, tag="gfd")
            nc.sync.dma_start(out=gfin_d, in_=gfin)
            pcs = spool.tile([P, 2 * CT], F32, tag="pcs")
            for g in range(4):
                nc.sync.dma_start(
                    out=pcs[32 * g : 32 * (g + 1), :],
                    in_=gfin_d[g : g + 1, :].to_broadcast((32, 2 * CT)),
                )
            h_t = hpool.tile([P, CT, N], F8, tag="h")
            st[b]["h"] = h_t
            for ct in range(CT):
                nc.vector.tensor_scalar(
                    out=h_t[:, ct, :],
                    in0=x_t[:, ct, :],
                    scalar1=pcs[:, 2 * ct : 2 * ct + 1],
                    scalar2=pcs[:, 2 * ct + 1 : 2 * ct + 2],
                    op0=OP.subtract,
                    op1=OP.mult,
                )

        def phase_b(b):
            """qkv 1x1 convs (fp8 DoubleRow, full C contraction per mm)."""
            h_t = st[b]["h"]
            q_sb = qpool.tile([P, CT, N], F8, tag="q")
            k_sb = kpool.tile([P, CT, N], F8, tag="k")
            st[b]["q"], st[b]["k"] = q_sb, k_sb
            for dst, w_sb, b_sb, use_b, on_act in (
                (q_sb, wq_sb, bq_sb, use_bq, True),
                (k_sb, wk_sb, bk_sb, use_bk, False),
            ):
                for ct in range(CT):
                    for nch in range(2):
                        mm_ps = psw.tile([P, 512], F32, tag="w")
                        nc.tensor.matmul(
                            mm_ps,
                            lhsT=w_sb[:, :, ct * P : (ct + 1) * P],
                            rhs=h_t[:, :, nch * 512 : (nch + 1) * 512],
                            start=True,
                            stop=True,
                            perf_mode=DR,
                        )
                        dst_ap = dst[:, ct, nch * 512 : (nch + 1) * 512]
                        if use_b:
                            nc.vector.tensor_scalar_add(
                                out=dst_ap, in0=mm_ps, scalar1=b_sb[:, ct : ct + 1]
                            )
                        elif on_act:
                            nc.scalar.activation(
                                out=dst_ap, in_=mm_ps, func=AF.Copy, bias=0.0,
                                scale=1.0,
                            )
                        else:
                            nc.vector.tensor_copy(out=dst_ap, in_=mm_ps)
            v_sb = vpool.tile([P, NB, C], F8, tag="v")
            st[b]["v"] = v_sb
            for nb in range(NB):
                vv_ps = psw.tile([P, C], F32, tag="w")
                nc.tensor.matmul(
                    vv_ps,
                    lhsT=h_t[:, :, nb * P : (nb + 1) * P],
                    rhs=wv_sb[:, :, :],
                    start=True,
                    stop=True,
                    perf_mode=DR,
                )
                nc.vector.tensor_copy(out=v_sb[:, nb, :], in_=vv_ps)

        def phase_c(b, qc):
            """Attention core for one 512-wide q chunk: S, exp, Z, O."""
            q_sb, k_sb, v_sb = st[b]["q"], st[b]["k"], st[b]["v"]
            O_ps = psO.tile([P, CT, 512], F32, tag="O")
            z_ps = psz.tile([1, 512], F32, tag="z")
            st[b]["zps%d" % qc] = z_ps

            def s_matmul(nb):
                s_ps = psw.tile([P, 512], F32, tag="w", name="s_ps")
                nc.tensor.matmul(
                    s_ps,
                    lhsT=k_sb[:, :, nb * P : (nb + 1) * P],
                    rhs=q_sb[:, :, qc * 512 : (qc + 1) * 512],
                    start=True,
                    stop=True,
                    perf_mode=DR,
                )
                return s_ps

            # Deep software pipeline: the S fifo keeps S_FIFO tiles in
            # flight so the exp-gated Z/O pair matmuls rarely stall the
            # PE on ScalarE.
            s_fifo = [s_matmul(i) for i in range(S_FIFO)]
            nxt = S_FIFO
            p_cur = None
            for nb in range(NB):
                j, t = divmod(nb, 2)
                if t == 0:
                    p_cur = ppool.tile([P, 2, 512], F8, tag="p")
                s_ps = s_fifo.pop(0)
                if nxt < NB:
                    s_fifo.append(s_matmul(nxt))
                    nxt += 1
                nc.scalar.activation(
                    out=p_cur[:, t, :], in_=s_ps, func=AF.Exp,
                    bias=shift_sb, scale=SCALE,
                )
                if t == 1:
                    nc.tensor.matmul(
                        z_ps,
                        lhsT=onesp_sb[:, :, 0:1],
                        rhs=p_cur,
                        start=(j == 0),
                        stop=(j == NP - 1),
                        perf_mode=DR,
                    )
                    for ct in range(CT):
                        nc.tensor.matmul(
                            O_ps[:, ct, :],
                            lhsT=v_sb[:, 2 * j : 2 * j + 2, ct * P : (ct + 1) * P],
                            rhs=p_cur,
                            start=(j == 0),
                            stop=(j == NP - 1),
                            perf_mode=DR,
                        )
            # proj commutes with the per-column 1/Z scale, so proj depends
            # only on O: cast O out of PSUM here (releasing the O banks a
            # chunk early); the 1/Z bounce gates just the final DVE op.
            on_sb = opool.tile([P, CT, 512], F8, tag="on")
            st[b]["on%d" % qc] = on_sb
            for ct in range(CT):
                nc.vector.tensor_copy(out=on_sb[:, ct, :], in_=O_ps[:, ct, :])

        def phase_rz(b, qc):
            z_ps = st[b].pop("zps%d" % qc)
            # 1/Z with the row transposed to [128, 4] so the reciprocal
            # runs across lanes (a [1, 512] reciprocal costs ~4us on DVE).
            z_sb = rzpool.tile([1, 512], F32, tag="zsb")
            nc.vector.tensor_copy(out=z_sb, in_=z_ps)
            z_d = dram.tile([1, 512], F32, tag="zd")
            nc.sync.dma_start(out=z_d, in_=z_sb)
            zT_sb = rzpool.tile([P, 4], F32, tag="zT")
            nc.sync.dma_start(
                out=zT_sb, in_=z_d[0, :].rearrange("(p j) -> p j", j=4)
            )
            rzT_sb = rzpool.tile([P, 4], F32, tag="rzT")
            nc.vector.reciprocal(out=rzT_sb, in_=zT_sb)
            rz_d = dram.tile([1, 512], F32, tag="rzd")
            nc.sync.dma_start(
                out=rz_d[0, :].rearrange("(p j) -> p j", j=4), in_=rzT_sb
            )
            rzb_sb = rzpool.tile([P, 512], F32, tag="rzb")
            st[b]["rzb%d" % qc] = rzb_sb
            nc.sync.dma_start(out=rzb_sb, in_=rz_d[:, :].to_broadcast((P, 512)))

        def phase_d(b, qc):
            """Apply 1/Z, proj conv, residual add, store."""
            rzb_sb = st[b].pop("rzb%d" % qc)
            x_t = st[b]["x"]
            on_sb = st[b].pop("on%d" % qc)
            for ct in range(CT):
                pr_ps = psw.tile([P, 512], F32, tag="w")
                nc.tensor.matmul(
                    pr_ps,
                    lhsT=wp_sb[:, :, ct * P : (ct + 1) * P],
                    rhs=on_sb[:, :, :],
                    start=True,
                    stop=True,
                    perf_mode=DR,
                )
                o_sb = outp.tile([P, 512], F32, tag="o")
                xres = x_t[:, ct, qc * 512 : (qc + 1) * 512]
                nc.vector.tensor_mul(out=o_sb, in0=pr_ps, in1=rzb_sb)
                if use_bf:
                    nc.vector.scalar_tensor_tensor(
                        out=o_sb,
                        in0=o_sb,
                        scalar=bf_sb[:, ct : ct + 1],
                        in1=xres,
                        op0=OP.add,
                        op1=OP.add,
                    )
                else:
                    nc.vector.tensor_add(out=o_sb, in0=o_sb, in1=xres)
                nc.sync.dma_start(
                    out=out[b, ct * P : (ct + 1) * P, qc * 512 : (qc + 1) * 512],
                    in_=o_sb,
                )

        # Software pipeline: hide the stats chain of image b+1 under the
        # attention of image b, and each q-chunk's 1/Z DRAM bounce under
        # the next chunk's matmuls.
        phase_a(0)
        load_weights()
        for _ in range(N_WARM):
            warm_ps = psw.tile([P, 512], F32, tag="w", name="warm_ps")
            nc.tensor.matmul(
                warm_ps[:, 0:256], lhsT=wq_sb[:, 0, 0:P],
                rhs=wq_sb[:, 0, 0:256], start=True, stop=True,
            )
        pending = None
        for b in range(BL):
            phase_b(b)
            if b + 1 < BL:
                phase_a(b + 1)
            for qc in range(QCH):
                phase_c(b, qc)
                if pending is not None:
                    phase_d(*pending)
                phase_rz(b, qc)
                pending = (b, qc)
        phase_d(*pending)
    nc.compile()
    return nc


def prepare(inputs):
    """Fold parameters on the host; return (program, per-core input maps)."""
    x = np.ascontiguousarray(np.asarray(inputs["x"], dtype=np.float32))
    norm_w = np.asarray(inputs["norm_w"], dtype=np.float32)
    norm_b = np.asarray(inputs["norm_b"], dtype=np.float32)
    qkv_w = np.asarray(inputs["qkv_w"], dtype=np.float32)
    qkv_b = np.asarray(inputs["qkv_b"], dtype=np.float32)
    proj_w = np.asarray(inputs["proj_w"], dtype=np.float32)
    proj_b = np.asarray(inputs["proj_b"], dtype=np.float32)

    # Fold the GroupNorm affine into qkv: qkv(h*w+b) = (qkv*w)h + qkv@b
    w_eff = qkv_w * norm_w[None, :]
    b_eff = qkv_b + qkv_w @ norm_b
    fp8 = ml_dtypes.float8_e4m3

    def to8(a):
        return np.ascontiguousarray(np.clip(a, -240.0, 240.0).astype(fp8))

    # q/k stay unscaled (~N(0,1) is the fp8 sweet spot); the attention
    # 1/sqrt(C) scale is applied inside the Exp activation on-chip.
    wq_t = to8(w_eff[0:C].T)
    wk_t = to8(w_eff[C : 2 * C].T)
    wv_t = to8(w_eff[2 * C : 3 * C].T)
    wp_t = to8(proj_w.T)
    bq_f = np.ascontiguousarray(b_eff[0:C])
    bk_f = np.ascontiguousarray(b_eff[C : 2 * C])
    bv_f = b_eff[2 * C : 3 * C]
    bf_f = np.ascontiguousarray(proj_w @ bv_f + proj_b)

    use_bq = bool(np.any(bq_f))
    use_bk = bool(np.any(bk_f))
    use_bf = bool(np.any(bf_f))
    nc = build_program(use_bq, use_bk, use_bf)

    xr = x.reshape(NCORES, BL, C, N)
    in_maps = []
    for c in range(NCORES):
        in_maps.append(
            {
                "xs": np.ascontiguousarray(xr[c]),
                "wq": wq_t,
                "wk": wk_t,
                "wv": wv_t,
                "wp": wp_t,
                "bq": bq_f,
                "bk": bk_f,
                "bf": bf_f,
            }
        )
    return nc, in_maps


def run(inputs, trace=False):
    from concourse.bass_utils import run_bass_kernel_spmd

    nc, in_maps = prepare(inputs)
    res = run_bass_kernel_spmd(nc, in_maps, list(range(NCORES)), trace=trace)
    outs = np.stack([np.asarray(res.results[i]["out"]) for i in range(NCORES)])
    full = outs.reshape(B, C, H, W).astype(np.float32)
    return full, res


def kernel(**inputs) -> np.ndarray:
    full, _ = run(inputs, trace=False)
    return full


# revision 47
# speedup vs baseline: 1.2191x; 1.1569x over previous
"""AttentionBlock Trainium2 kernel (Bass/Tile), SPMD over 8 NeuronCores.

Problem (hardcoded): x [32, 256, 32, 32] fp32
  GroupNorm(8 groups, eps=1e-5, affine) -> 1x1 qkv conv [768,256] ->
  per-image attention over N=1024 pixels (C=256) -> 1x1 proj [256,256] ->
  residual add.

Sharding: pure data-parallel over batch: 4 images per core, weights
replicated, no collectives.

All matmuls run in fp8(e4m3) with DoubleRow perf mode: the PE array
virtualizes to 128x256, so a full C=256 (or 2-k-block) contraction is
ONE matmul at 2 MAC/cell/cycle (~259 ns warm for a 512-wide output vs
~2x426 ns in bf16). Layouts keep channels split as [c_lo(partition),
c_hi(2), ...] so every operand slices directly into the required
[Ki, 2, free] 3D access pattern:
  - h/q/k stored [P, CT, N] fp8; weights [P, CT, C] fp8; v [P, NB, C] fp8.
  - S^T[k,q] = k^T q unscaled (q,k keep ~N(0,1) range for fp8); the
    1/sqrt(C) scale and a -3 shift fold into the Exp activation
    (exp(S/16 - 3)), keeping exp outputs < 16 (fp8e4 max 240, inf above)
    while the shift cancels exactly between O and Z.
  - exp writes fp8 k-block pairs p[P, 2, 512]; O and the Z row
    accumulate over 4 pair-matmuls per 512-wide q chunk in PSUM.
  - GroupNorm stats via bn_stats/bn_aggr (per-channel, fp32), pooled
    over each group's 32 channels with a tiny bf16 mask-matmul,
    finalized at group level in fp32, broadcast back to channels via a
    DRAM-bounce DMA.
  - 1/Z via a [128,4]-transposed reciprocal (DRAM bounce; the direct
    SBUF->SBUF redistribute silently misroutes on hardware); proj
    commutes with the per-q-column 1/Z scale, so proj runs directly on
    the fp8 O and the bounce only gates the final DVE multiply-add:
    y = x + proj(O) * (1/Z) + bias.

The per-image work is software-pipelined at emission order (per-engine
instruction streams execute in order): image b+1's x-load/stats run on
DVE under image b's attention matmuls, the S fifo keeps 3 tiles in
flight so the PE rarely waits on ScalarE's exp, each q-chunk's 1/Z DRAM
bounce hides under the next chunk's matmuls (phase_d deferred by one
chunk), and the 4-deep single-bank psum ring for S/qkv/proj keeps slot
antidependencies loose. Dep-free warm-up matmuls after the weight DMAs
un-throttle the PE clock (HAM) before the first real compute.

Measured on 8 axon trn2 cores: ~153-190 us HW exec (run-to-run HAM/
scheduling variance), rel err ~6.5e-3.
"""

from contextlib import ExitStack

import ml_dtypes
import numpy as np

import concourse.bass as bass
import concourse.tile as tile
from concourse import bacc
from concourse import mybir

F32 = mybir.dt.float32
BF16 = mybir.dt.bfloat16
F8 = mybir.dt.float8e4
AF = mybir.ActivationFunctionType
OP = mybir.AluOpType
DR = mybir.MatmulPerfMode.DoubleRow

B, C, H, W = 32, 256, 32, 32
N = H * W            # 1024
G = 8                # groups
EPS = 1e-5
NCORES = 8
BL = B // NCORES     # images per core
CT = C // 128        # channel tiles
NB = N // 128        # pixel blocks (k dim of attention)
NP = NB // 2         # k-block pairs per q chunk
QCH = N // 512       # 512-wide q chunks
P = 128
SHIFT = 3.0          # exp(S/16 - SHIFT): keeps fp8 exp outputs < 16
SCALE = float(C) ** -0.5
import os as _os
N_WARM = int(_os.environ.get("KERNEL_N_WARM", "24"))
S_FIFO = int(_os.environ.get("KERNEL_S_FIFO", "3"))


def build_program(use_bq: bool, use_bk: bool, use_bf: bool) -> bass.Bass:
    nc = bacc.Bacc()

    xs = nc.dram_tensor("xs", [BL, C, N], F32, kind="ExternalInput")
    wq = nc.dram_tensor("wq", [C, C], F8, kind="ExternalInput")  # [c_in, c_out]
    wk = nc.dram_tensor("wk", [C, C], F8, kind="ExternalInput")
    wv = nc.dram_tensor("wv", [C, C], F8, kind="ExternalInput")
    wp = nc.dram_tensor("wp", [C, C], F8, kind="ExternalInput")
    bq = nc.dram_tensor("bq", [C], F32, kind="ExternalInput")
    bk = nc.dram_tensor("bk", [C], F32, kind="ExternalInput")
    bf = nc.dram_tensor("bf", [C], F32, kind="ExternalInput")
    out = nc.dram_tensor("out", [BL, C, N], F32, kind="ExternalOutput")

    # Constant matrix for the group-stat pooling matmul (mean over each
    # group's 32 channels; 1/32 is exact in bf16).
    gmask_np = np.zeros((P, 4), np.float32)
    gmask_np[np.arange(P), np.arange(P) // 32] = 1.0 / 32.0
    gmask_d = nc.inline_tensor(gmask_np.astype(ml_dtypes.bfloat16), "gmask")

    with tile.TileContext(nc) as tc, ExitStack() as ctx:
        consts = ctx.enter_context(tc.tile_pool(name="consts", bufs=1))
        xpool = ctx.enter_context(tc.tile_pool(name="xp", bufs=3))
        hpool = ctx.enter_context(tc.tile_pool(name="hp", bufs=2))
        qpool = ctx.enter_context(tc.tile_pool(name="qp", bufs=2))
        kpool = ctx.enter_context(tc.tile_pool(name="kp", bufs=2))
        vpool = ctx.enter_context(tc.tile_pool(name="vp", bufs=2))
        ppool = ctx.enter_context(tc.tile_pool(name="pp", bufs=3))
        opool = ctx.enter_context(tc.tile_pool(name="op", bufs=2))
        spool = ctx.enter_context(tc.tile_pool(name="sp", bufs=2))
        rzpool = ctx.enter_context(tc.tile_pool(name="rzp", bufs=2))
        outp = ctx.enter_context(tc.tile_pool(name="outp", bufs=4))
        dram = ctx.enter_context(tc.tile_pool(name="dram", bufs=2, space="DRAM"))
        psw = ctx.enter_context(tc.tile_pool(name="psw", bufs=4, space="PSUM"))
        psO = ctx.enter_context(tc.tile_pool(name="psO", bufs=1, space="PSUM"))
        psz = ctx.enter_context(tc.tile_pool(name="psz", bufs=2, space="PSUM"))

        # --- constants ---
        gmask_sb = consts.tile([P, 4], BF16, tag="gmask")
        nc.sync.dma_start(out=gmask_sb, in_=gmask_d[:, :])
        bq_sb = consts.tile([P, CT], F32, tag="bq")
        nc.sync.dma_start(out=bq_sb, in_=bq[:].rearrange("(t p) -> p t", p=P))
        bk_sb = consts.tile([P, CT], F32, tag="bk")
        nc.sync.dma_start(out=bk_sb, in_=bk[:].rearrange("(t p) -> p t", p=P))
        bf_sb = consts.tile([P, CT], F32, tag="bf")
        nc.sync.dma_start(out=bf_sb, in_=bf[:].rearrange("(t p) -> p t", p=P))
        # fp8 ones pairs for the Z row matmul; 16-wide so the middle-dim
        # byte stride satisfies the DoubleRow AP constraint (step%16==0).
        onesp_sb = consts.tile([P, 2, 16], F8, tag="onesp")
        nc.vector.memset(onesp_sb, 1.0)
        eps_sb = consts.tile([P, 1], F32, tag="eps")
        nc.vector.memset(eps_sb, EPS)
        shift_sb = consts.tile([P, 1], F32, tag="shift")
        nc.vector.memset(shift_sb, -SHIFT)
        wq_sb = consts.tile([P, CT, C], F8, tag="wq")
        wk_sb = consts.tile([P, CT, C], F8, tag="wk")
        wv_sb = consts.tile([P, CT, C], F8, tag="wv")
        wp_sb = consts.tile([P, CT, C], F8, tag="wp")

        def load_weights():
            for t_sb, t_d in ((wq_sb, wq), (wk_sb, wk), (wv_sb, wv), (wp_sb, wp)):
                nc.sync.dma_start(
                    out=t_sb, in_=t_d[:, :].rearrange("(t p) o -> p t o", p=P)
                )

        # Per-image state carried between pipeline phases.
        st = [dict() for _ in range(BL)]

        def phase_a(b):
            """Load x, GroupNorm stats -> per-channel (mean, rstd), h."""
            x_t = xpool.tile([P, CT, N], F32, tag="x")
            st[b]["x"] = x_t
            # Bulk loads ride the (otherwise idle) GpSimd DMA queue so the
            # latency-critical pcs/1Z bounce DMAs never queue behind them
            # on the sync engine.
            for ct in range(CT):
                nc.gpsimd.dma_start(
                    out=x_t[:, ct, :], in_=xs[b, ct * P : (ct + 1) * P, :]
                )
            chst = spool.tile([P, 2 * CT], F32, tag="chst")
            for ct in range(CT):
                bnst = spool.tile([P, 2, 6], F32, tag="bnst")
                for s in range(2):
                    nc.vector.bn_stats(
                        out=bnst[:, s, :], in_=x_t[:, ct, s * 512 : (s + 1) * 512]
                    )
                nc.vector.bn_aggr(out=chst[:, 2 * ct : 2 * ct + 2], in_=bnst)
                msq = spool.tile([P, 1], F32, tag="msq")
                nc.vector.tensor_mul(
                    out=msq,
                    in0=chst[:, 2 * ct : 2 * ct + 1],
                    in1=chst[:, 2 * ct : 2 * ct + 1],
                )
                nc.vector.tensor_add(
                    out=chst[:, 2 * ct + 1 : 2 * ct + 2],
                    in0=chst[:, 2 * ct + 1 : 2 * ct + 2],
                    in1=msq,
                )
            chst_bf = spool.tile([P, 2 * CT], BF16, tag="chstbf")
            nc.vector.tensor_copy(out=chst_bf, in_=chst)
            gst_ps = psw.tile([4, 2 * CT], F32, tag="w")
            nc.tensor.matmul(
                gst_ps, lhsT=gmask_sb, rhs=chst_bf, start=True, stop=True
            )
            gst_sb = spool.tile([4, 2 * CT], F32, tag="gst")
            nc.vector.tensor_copy(out=gst_sb, in_=gst_ps)
            gvar = spool.tile([4, CT], F32, tag="gvar")
            for ct in range(CT):
                gmsq = spool.tile([4, 1], F32, tag="gmsq")
                nc.vector.tensor_mul(
                    out=gmsq,
                    in0=gst_sb[:, 2 * ct : 2 * ct + 1],
                    in1=gst_sb[:, 2 * ct : 2 * ct + 1],
                )
                nc.vector.tensor_tensor(
                    out=gvar[:, ct : ct + 1],
                    in0=gst_sb[:, 2 * ct + 1 : 2 * ct + 2],
                    in1=gmsq,
                    op=OP.subtract,
                )
            gsd = spool.tile([4, CT], F32, tag="gsd")
            nc.scalar.activation(
                out=gsd, in_=gvar, func=AF.Sqrt, bias=eps_sb[0:4], scale=1.0
            )
            # Dummy exp right after the Sqrt: both ACT table reloads land
            # in this idle window instead of ahead of the next chunk's
            # first real exp (a table swap costs ~1.5us on ScalarE).
            dummy = spool.tile([P, 1], F32, tag="dummy")
            nc.scalar.activation(
                out=dummy, in_=shift_sb, func=AF.Exp, bias=shift_sb, scale=1.0
            )
            grstd = spool.tile([4, CT], F32, tag="grstd")
            nc.vector.reciprocal(out=grstd, in_=gsd)
            gfin = spool.tile([4, 2 * CT], F32, tag="gfin")
            for ct in range(CT):
                nc.vector.tensor_copy(
                    out=gfin[:, 2 * ct : 2 * ct + 1],
                    in_=gst_sb[:, 2 * ct : 2 * ct + 1],
                )
                nc.vector.tensor_copy(
                    out=gfin[:, 2 * ct + 1 : 2 * ct + 2],
                    in_=grstd[:, ct : ct + 1],
                )
            gfin_d = dram.tile([4, 2 * CT], F32, tag="gfd")
            nc.sync.dma_start(out=gfin_d, in_=gfin)
            pcs = spool.tile([P, 2 * CT], F32, tag="pcs")
            for g in range(4):
                nc.sync.dma_start(
                    out=pcs[32 * g : 32 * (g + 1), :],
                    in_=gfin_d[g : g + 1, :].to_broadcast((32, 2 * CT)),
                )
            h_t = hpool.tile([P, CT, N], F8, tag="h")
            st[b]["h"] = h_t
            for ct in range(CT):
                nc.vector.tensor_scalar(
                    out=h_t[:, ct, :],
                    in0=x_t[:, ct, :],
                    scalar1=pcs[:, 2 * ct : 2 * ct + 1],
                    scalar2=pcs[:, 2 * ct + 1 : 2 * ct + 2],
                    op0=OP.subtract,
                    op1=OP.mult,
                )

        def phase_b(b):
            """qkv 1x1 convs (fp8 DoubleRow, full C contraction per mm)."""
            h_t = st[b]["h"]
            q_sb = qpool.tile([P, CT, N], F8, tag="q")
            k_sb = kpool.tile([P, CT, N], F8, tag="k")
            st[b]["q"], st[b]["k"] = q_sb, k_sb
            for dst, w_sb, b_sb, use_b, on_act in (
                (q_sb, wq_sb, bq_sb, use_bq, True),
                (k_sb, wk_sb, bk_sb, use_bk, False),
            ):
                for ct in range(CT):
                    for nch in range(2):
                        mm_ps = psw.tile([P, 512], F32, tag="w")
                        nc.tensor.matmul(
                            mm_ps,
                            lhsT=w_sb[:, :, ct * P : (ct + 1) * P],
                            rhs=h_t[:, :, nch * 512 : (nch + 1) * 512],
                            start=True,
                            stop=True,
                            perf_mode=DR,
                        )
                        dst_ap = dst[:, ct, nch * 512 : (nch + 1) * 512]
                        if use_b:
                            nc.vector.tensor_scalar_add(
                                out=dst_ap, in0=mm_ps, scalar1=b_sb[:, ct : ct + 1]
                            )
                        elif on_act:
                            nc.scalar.activation(
                                out=dst_ap, in_=mm_ps, func=AF.Copy, bias=0.0,
                                scale=1.0,
                            )
                        else:
                            nc.vector.tensor_copy(out=dst_ap, in_=mm_ps)
            v_sb = vpool.tile([P, NB, C], F8, tag="v")
            st[b]["v"] = v_sb
            for nb in range(NB):
                vv_ps = psw.tile([P, C], F32, tag="w")
                nc.tensor.matmul(
                    vv_ps,
                    lhsT=h_t[:, :, nb * P : (nb + 1) * P],
                    rhs=wv_sb[:, :, :],
                    start=True,
                    stop=True,
                    perf_mode=DR,
                )
                nc.vector.tensor_copy(out=v_sb[:, nb, :], in_=vv_ps)

        def phase_c(b, qc):
            """Attention core for one 512-wide q chunk: S, exp, Z, O."""
            q_sb, k_sb, v_sb = st[b]["q"], st[b]["k"], st[b]["v"]
            O_ps = psO.tile([P, CT, 512], F32, tag="O")
            z_ps = psz.tile([1, 512], F32, tag="z")
            st[b]["zps%d" % qc] = z_ps

            def s_matmul(nb):
                s_ps = psw.tile([P, 512], F32, tag="w", name="s_ps")
                nc.tensor.matmul(
                    s_ps,
                    lhsT=k_sb[:, :, nb * P : (nb + 1) * P],
                    rhs=q_sb[:, :, qc * 512 : (qc + 1) * 512],
                    start=True,
                    stop=True,
                    perf_mode=DR,
                )
                return s_ps

            # Deep software pipeline: the S fifo keeps S_FIFO tiles in
            # flight so the exp-gated Z/O pair matmuls rarely stall the
            # PE on ScalarE.
            s_fifo = [s_matmul(i) for i in range(S_FIFO)]
            nxt = S_FIFO
            p_cur = None
            for nb in range(NB):
                j, t = divmod(nb, 2)
                if t == 0:
                    p_cur = ppool.tile([P, 2, 512], F8, tag="p")
                s_ps = s_fifo.pop(0)
                if nxt < NB:
                    s_fifo.append(s_matmul(nxt))
                    nxt += 1
                nc.scalar.activation(
                    out=p_cur[:, t, :], in_=s_ps, func=AF.Exp,
                    bias=shift_sb, scale=SCALE,
                )
                if t == 1:
                    nc.tensor.matmul(
                        z_ps,
                        lhsT=onesp_sb[:, :, 0:1],
                        rhs=p_cur,
                        start=(j == 0),
                        stop=(j == NP - 1),
                        perf_mode=DR,
                    )
                    for ct in range(CT):
                        nc.tensor.matmul(
                            O_ps[:, ct, :],
                            lhsT=v_sb[:, 2 * j : 2 * j + 2, ct * P : (ct + 1) * P],
                            rhs=p_cur,
                            start=(j == 0),
                            stop=(j == NP - 1),
                            perf_mode=DR,
                        )
            # proj commutes with the per-column 1/Z scale, so proj depends
            # only on O: cast O out of PSUM here (releasing the O banks a
            # chunk early); the 1/Z bounce gates just the final DVE op.
            on_sb = opool.tile([P, CT, 512], F8, tag="on")
            st[b]["on%d" % qc] = on_sb
            for ct in range(CT):
                nc.vector.tensor_copy(out=on_sb[:, ct, :], in_=O_ps[:, ct, :])

        def phase_rz(b, qc):
            z_ps = st[b].pop("zps%d" % qc)
            # 1/Z with the row transposed to [128, 4] so the reciprocal
            # runs across lanes (a [1, 512] reciprocal costs ~4us on DVE).
            z_sb = rzpool.tile([1, 512], F32, tag="zsb")
            nc.vector.tensor_copy(out=z_sb, in_=z_ps)
            z_d = dram.tile([1, 512], F32, tag="zd")
            nc.sync.dma_start(out=z_d, in_=z_sb)
            zT_sb = rzpool.tile([P, 4], F32, tag="zT")
            nc.sync.dma_start(
                out=zT_sb, in_=z_d[0, :].rearrange("(p j) -> p j", j=4)
            )
            rzT_sb = rzpool.tile([P, 4], F32, tag="rzT")
            nc.vector.reciprocal(out=rzT_sb, in_=zT_sb)
            rz_d = dram.tile([1, 512], F32, tag="rzd")
            nc.sync.dma_start(
                out=rz_d[0, :].rearrange("(p j) -> p j", j=4), in_=rzT_sb
            )
            rzb_sb = rzpool.tile([P, 512], F32, tag="rzb")
            st[b]["rzb%d" % qc] = rzb_sb
            nc.sync.dma_start(out=rzb_sb, in_=rz_d[:, :].to_broadcast((P, 512)))

        def phase_d(b, qc):
            """Apply 1/Z, proj conv, residual add, store."""
            rzb_sb = st[b].pop("rzb%d" % qc)
            x_t = st[b]["x"]
            on_sb = st[b].pop("on%d" % qc)
            for ct in range(CT):
                pr_ps = psw.tile([P, 512], F32, tag="w")
                nc.tensor.matmul(
                    pr_ps,
                    lhsT=wp_sb[:, :, ct * P : (ct + 1) * P],
                    rhs=on_sb[:, :, :],
                    start=True,
                    stop=True,
                    perf_mode=DR,
                )
                o_sb = outp.tile([P, 512], F32, tag="o")
                xres = x_t[:, ct, qc * 512 : (qc + 1) * 512]
                nc.vector.tensor_mul(out=o_sb, in0=pr_ps, in1=rzb_sb)
                if use_bf:
                    nc.vector.scalar_tensor_tensor(
                        out=o_sb,
                        in0=o_sb,
                        scalar=bf_sb[:, ct : ct + 1],
                        in1=xres,
                        op0=OP.add,
                        op1=OP.add,
                    )
                else:
                    nc.vector.tensor_add(out=o_sb, in0=o_sb, in1=xres)
                nc.gpsimd.dma_start(
                    out=out[b, ct * P : (ct + 1) * P, qc * 512 : (qc + 1) * 512],
                    in_=o_sb,
                )

        # Software pipeline: hide the stats chain of image b+1 under the
        # attention of image b, and each q-chunk's 1/Z DRAM bounce under
        # the next chunk's matmuls.
        phase_a(0)
        load_weights()
        for _ in range(N_WARM):
            warm_ps = psw.tile([P, 512], F32, tag="w", name="warm_ps")
            nc.tensor.matmul(
                warm_ps[:, 0:256], lhsT=wq_sb[:, 0, 0:P],
                rhs=wq_sb[:, 0, 0:256], start=True, stop=True,
            )
        pending = None
        for b in range(BL):
            phase_b(b)
            if b + 1 < BL:
                phase_a(b + 1)
            for qc in range(QCH):
                phase_c(b, qc)
                if pending is not None:
                    phase_d(*pending)
                phase_rz(b, qc)
                pending = (b, qc)
        phase_d(*pending)
    nc.compile()
    return nc


def prepare(inputs):
    """Fold parameters on the host; return (program, per-core input maps)."""
    x = np.ascontiguousarray(np.asarray(inputs["x"], dtype=np.float32))
    norm_w = np.asarray(inputs["norm_w"], dtype=np.float32)
    norm_b = np.asarray(inputs["norm_b"], dtype=np.float32)
    qkv_w = np.asarray(inputs["qkv_w"], dtype=np.float32)
    qkv_b = np.asarray(inputs["qkv_b"], dtype=np.float32)
    proj_w = np.asarray(inputs["proj_w"], dtype=np.float32)
    proj_b = np.asarray(inputs["proj_b"], dtype=np.float32)

    # Fold the GroupNorm affine into qkv: qkv(h*w+b) = (qkv*w)h + qkv@b
    w_eff = qkv_w * norm_w[None, :]
    b_eff = qkv_b + qkv_w @ norm_b
    fp8 = ml_dtypes.float8_e4m3

    def to8(a):
        return np.ascontiguousarray(np.clip(a, -240.0, 240.0).astype(fp8))

    # q/k stay unscaled (~N(0,1) is the fp8 sweet spot); the attention
    # 1/sqrt(C) scale is applied inside the Exp activation on-chip.
    wq_t = to8(w_eff[0:C].T)
    wk_t = to8(w_eff[C : 2 * C].T)
    wv_t = to8(w_eff[2 * C : 3 * C].T)
    wp_t = to8(proj_w.T)
    bq_f = np.ascontiguousarray(b_eff[0:C])
    bk_f = np.ascontiguousarray(b_eff[C : 2 * C])
    bv_f = b_eff[2 * C : 3 * C]
    bf_f = np.ascontiguousarray(proj_w @ bv_f + proj_b)

    use_bq = bool(np.any(bq_f))
    use_bk = bool(np.any(bk_f))
    use_bf = bool(np.any(bf_f))
    nc = build_program(use_bq, use_bk, use_bf)

    xr = x.reshape(NCORES, BL, C, N)
    in_maps = []
    for c in range(NCORES):
        in_maps.append(
            {
                "xs": np.ascontiguousarray(xr[c]),
                "wq": wq_t,
                "wk": wk_t,
                "wv": wv_t,
                "wp": wp_t,
                "bq": bq_f,
                "bk": bk_f,
                "bf": bf_f,
            }
        )
    return nc, in_maps


def run(inputs, trace=False):
    from concourse.bass_utils import run_bass_kernel_spmd

    nc, in_maps = prepare(inputs)
    res = run_bass_kernel_spmd(nc, in_maps, list(range(NCORES)), trace=trace)
    outs = np.stack([np.asarray(res.results[i]["out"]) for i in range(NCORES)])
    full = outs.reshape(B, C, H, W).astype(np.float32)
    return full, res


def kernel(**inputs) -> np.ndarray:
    full, _ = run(inputs, trace=False)
    return full
